# revision 17
# baseline (speedup 1.0000x reference)
"""Trainium2 Bass kernel for MetaDynamics potential evaluation.

out[p] = sum_h hgt[h] * exp(-0.5 * sum_d (cen[h,d]-col[p,d])^2 / wdt[h,d]^2)
with H=16384 hills, P=4096 points, D=8 collective variables.

Algorithm: expand the quadratic form into a rank-17 inner product and fold an
affine map into the weights so the PE emits y = A*e' + B directly:
  e'[h,p] = sum_d (cen*c)[h,d]*col[p,d] - 0.5*sum_d c[h,d]*col[p,d]^2 - 0.5*a[h]
  c = 1/wdt^2, a[h] = sum_d cen^2*c - 2*ln(hgt[h]);   out[p] = sum_h exp(e'[h,p])
  A = 2^23/ln2, B = 127*2^23 - 486411
Both factors are split into bf16 hi+lo parts and stacked to K=51 which
reproduces the fp32 product to ~2^-18 relative at full bf16 PE rate.

Sharding: hills are split across the 8 NeuronCores (2048 each); every core
computes a partial [4096] potential and the host sums the partials.

Per-core dataflow: the exp+sum work is split across TWO engines so the Scalar
(ACT) engine is no longer the sole bottleneck:
  for each of 32 point-tiles (128 points):
    4x matmul [K=51, M=128, N=512] -> two [128, 1024] PSUM tiles y = A*e'+B
    ACT tiles (21): 2x ACTIVATE(Exp, scale=1/A, bias=-B/A) in-place with
       accum_out -> exact exp + hill-sum on the Scalar engine
    DVE tiles (11): Schraudolph exp on the Vector engine:
       2x tensor_scalar(max,0) converts y -> int32 in SBUF (relu kills
       underflowed hills: bitcast(0)=0.0), then the int32 tile REINTERPRETED
       as f32 is ~exp(e') (mean-zero sawtooth err ~1.7% RMS, cancels over the
       2048-hill sum); one tensor_reduce(add) gives the [128,1] partial.
  PSUM is divided into 4 rotating [128, 1024] buffers (2 banks each) and every
  consumer instruction covers exactly one buffer, giving a depth-4 software
  pipeline: the PE never stalls on a consumer more than 4 half-tiles behind.
  Host picks each tile's column(s) from the matching accumulator, sums cores.
"""

import numpy as np
import ml_dtypes

import concourse.bacc as bacc
import concourse.mybir as mybir
import concourse.tile as tile
from concourse import bass_utils

H, P, D = 16384, 4096, 8
NCORES = 8
HL = H // NCORES          # hills per core
K = 51                    # 3 x 17 stacked hi/lo blocks
PT = 128                  # points per tile (PSUM partitions)
NPT = P // PT             # 32 p-tiles
HC = 512                  # hills per matmul (one PSUM bank of f32)
NHC = HL // HC            # 4 matmuls per p-tile

SCHRAUDOLPH_A = 2.0**23 / np.log(2.0)
SCHRAUDOLPH_B = 1065353216.0 - 486411.0
# Vector-engine (Schraudolph) tiles; the rest go to the Scalar engine.
# Keep the last few tiles on ACT so the final (deferred) Vector reduce and the
# out_v DMA retire well before the kernel tail.
DVE_TILES = frozenset({2, 4, 7, 9, 12, 14, 17, 19, 22, 25, 27})

BF16 = mybir.dt.bfloat16
F32 = mybir.dt.float32
I32 = mybir.dt.int32

_NC_CACHE = None


def _build_nc():
    nc = bacc.Bacc(
        "TRN2",
        target_bir_lowering=False,
        debug=False,
        enable_asserts=False,
        num_devices=NCORES,
    )
    ft = nc.dram_tensor("ft", [K, P], BF16, kind="ExternalInput").ap()
    wt = nc.dram_tensor("wt", [K, HL], BF16, kind="ExternalInput").ap()
    # out_a[p_lane, 2*n_tile + half]: ACT path writes one accum column per
    # half-tile (host sums the pair); out_v[p_lane, n_tile] one per DVE tile.
    out_a = nc.dram_tensor("out_a", [PT, 2 * NPT], F32, kind="ExternalOutput").ap()
    out_v = nc.dram_tensor("out_v", [PT, NPT], F32, kind="ExternalOutput").ap()

    with tile.TileContext(nc) as tc:
        with (
            tc.tile_pool(name="const", bufs=1) as cpool,
            tc.tile_pool(name="ints", bufs=2) as ipool,
            tc.tile_pool(name="psum", bufs=4, space="PSUM") as ppool,
        ):
            ftt = cpool.tile([K, P], BF16)
            wtt = cpool.tile([K, HL], BF16)
            acc_a = cpool.tile([PT, 2 * NPT], F32)
            acc_v = cpool.tile([PT, NPT], F32)
            btile = cpool.tile([PT, 1], F32)
            nc.gpsimd.memset(btile[:], float(-SCHRAUDOLPH_B / SCHRAUDOLPH_A))

            # Critical-path loads, descriptor-posting parallelized across four
            # engine queues: MM(0) needs ft[:, 0:128] (lhsT) and wt[:, 0:512];
            # each is split into two partition-halves on separate queues so
            # the ~51-descriptor DIRECT2D writes overlap. The remaining wt
            # hill-chunks gate MM(1..3) progressively; ft's later point
            # columns can land late.
            nc.sync.dma_start(wtt[0:26, 0:HC], wt[0:26, 0:HC])
            nc.scalar.dma_start(wtt[26:K, 0:HC], wt[26:K, 0:HC])
            nc.scalar.dma_start(ftt[:, 0:PT], ft[:, 0:PT])
            nc.sync.dma_start(wtt[:, HC : 2 * HC], wt[:, HC : 2 * HC])
            nc.gpsimd.dma_start(wtt[:, 2 * HC : 3 * HC], wt[:, 2 * HC : 3 * HC])
            nc.gpsimd.dma_start(wtt[:, 3 * HC : HL], wt[:, 3 * HC : HL])
            nc.sync.dma_start(ftt[:, PT:1152], ft[:, PT:1152])
            nc.scalar.dma_start(ftt[:, 1152:2624], ft[:, 1152:2624])
            nc.sync.dma_start(ftt[:, 2624:P], ft[:, 2624:P])

            HH = HL // 2  # hills per half-tile (one [128, 1024] PSUM buffer)

            def emit_reduce(pending):
                it, d = pending
                nc.vector.tensor_reduce(
                    acc_v[:, d : d + 1],
                    it[:].bitcast(F32),
                    mybir.AxisListType.X,
                    mybir.AluOpType.add,
                )

            # The reduce of DVE tile d is deferred until after the next DVE
            # tile's converts: the converts are what free PSUM banks for the
            # PE, so they must never queue behind a 2.2us reduce.
            pending_red = None
            for i in range(NPT):
                lhsT = ftt[:, i * PT : (i + 1) * PT]
                if i in DVE_TILES:
                    it = ipool.tile([PT, HL], I32, name=f"it{i}")
                else:
                    it = None
                for h in range(2):
                    ph = ppool.tile([PT, HH], F32)  # 2 PSUM banks
                    for j in range(2):
                        off = (2 * h + j) * HC
                        nc.tensor.matmul(
                            ph[:, j * HC : (j + 1) * HC],
                            lhsT=lhsT,
                            rhs=wtt[:, off : off + HC],
                            start=True,
                            stop=True,
                        )
                    if i in DVE_TILES:
                        # Frees this buffer's banks as soon as the convert
                        # runs; the deferred reduce reads SBUF only.
                        nc.vector.tensor_scalar(
                            it[:, h * HH : (h + 1) * HH],
                            ph[:],
                            0.0,
                            None,
                            mybir.AluOpType.max,
                        )
                    else:
                        nc.scalar.activation(
                            ph[:],
                            ph[:],
                            mybir.ActivationFunctionType.Exp,
                            scale=float(1.0 / SCHRAUDOLPH_A),
                            bias=btile[:],
                            accum_out=acc_a[:, 2 * i + h : 2 * i + h + 1],
                        )
                if i in DVE_TILES:
                    if pending_red is not None:
                        emit_reduce(pending_red)
                    if i == max(DVE_TILES):
                        # Last DVE tile: no later converts need protecting, and
                        # running its reduce now keeps it off the kernel tail.
                        emit_reduce((it, i))
                        pending_red = None
                    else:
                        pending_red = (it, i)
                if i == NPT - 2:
                    nc.sync.dma_start(
                        out_a[:, : 2 * NPT - 2], acc_a[:, : 2 * NPT - 2]
                    )
            if pending_red is not None:
                emit_reduce(pending_red)
            nc.sync.dma_start(out_v[:], acc_v[:])
            nc.scalar.dma_start(out_a[:, 2 * NPT - 2 :], acc_a[:, 2 * NPT - 2 :])

    nc.compile()
    return nc


def _get_nc():
    global _NC_CACHE
    if _NC_CACHE is None:
        _NC_CACHE = _build_nc()
    return _NC_CACHE


def _split_bf16(x64):
    hi = x64.astype(ml_dtypes.bfloat16)
    lo = (x64 - hi.astype(np.float64)).astype(ml_dtypes.bfloat16)
    return hi, lo


def _prepare_inputs(col, cen, wdt, hgt):
    col64 = col.astype(np.float64)
    cen64 = cen.astype(np.float64)
    wdt64 = wdt.astype(np.float64)
    hgt64 = np.maximum(hgt.astype(np.float64), 1e-38)

    A, B = SCHRAUDOLPH_A, SCHRAUDOLPH_B
    c = 1.0 / (wdt64 * wdt64)                                     # [H, D]
    a = np.sum(cen64 * cen64 * c, axis=1) - 2.0 * np.log(hgt64)   # [H]
    W = np.concatenate(
        [cen64 * c * A, -0.5 * c * A, (-0.5 * a * A + B)[:, None]], axis=1
    )  # [H, 17]
    F = np.concatenate([col64, col64 * col64, np.ones((P, 1))], axis=1)   # [P, 17]

    Whi, Wlo = _split_bf16(W)
    Fhi, Flo = _split_bf16(F)

    ft = np.ascontiguousarray(np.concatenate([Fhi.T, Flo.T, Fhi.T], axis=0))  # [51, P]
    wt_full = np.concatenate([Whi.T, Whi.T, Wlo.T], axis=0)                   # [51, H]
    wts = [
        np.ascontiguousarray(wt_full[:, i * HL : (i + 1) * HL]) for i in range(NCORES)
    ]
    return ft, wts


def run_on_hw(col, cen, wdt, hgt, trace=False):
    """Run the SPMD kernel on 8 cores; returns (out[P] f32, BassKernelResults)."""
    ft, wts = _prepare_inputs(col, cen, wdt, hgt)
    nc = _get_nc()
    in_maps = [{"ft": ft, "wt": wts[i]} for i in range(NCORES)]
    res = bass_utils.run_bass_kernel_spmd(
        nc, in_maps, core_ids=list(range(NCORES)), trace=trace
    )
    col_sel = np.array([1 if i in DVE_TILES else 0 for i in range(NPT)])
    total = np.zeros(P, dtype=np.float64)
    for r in res.results:
        oa = r["out_a"].astype(np.float64)  # [PT, 2*NPT]
        ov = r["out_v"].astype(np.float64)  # [PT, NPT]
        act_sum = oa[:, 0::2] + oa[:, 1::2]  # [PT, NPT] halves combined
        merged = np.where(col_sel[None, :] == 1, ov, act_sum)  # [PT, NPT]
        total += merged.T.reshape(P)
    return total.astype(np.float32), res


def kernel(col, cen, wdt, hgt):
    out, _ = run_on_hw(col, cen, wdt, hgt, trace=False)
    return out
